# revision 24
# baseline (speedup 1.0000x reference)
"""Bass/Tile Trainium2 kernel for batched self-attention with diagonal
self-exclusion (LSA): out = softmax(mask_diag(Q K^T / t)) @ V.

Shapes: Q,K,V [64, 1024, 768] fp32, temperature [1] fp32.
Sharding: batch dim across 8 NeuronCores (8 batches/core, pure data parallel).

DMA-lean design (v2):
  - Host pre-casts Q,K,V to bf16 (the PE computes in bf16 anyway), halving
    HBM read traffic; output is stored bf16 and upcast on host.
  - Q,K are xbar DMA-transposed *directly from DRAM* into d-major SBUF
    tiles qT,kT [d, n] -- no staging tile, no SBUF->SBUF transpose pass.
  - S^T[k, q] = sum_d KT[d,k] * QT[d,q] on PE (bf16, fp32 PSUM accum),
    2 half-k groups x 2 q-halves x 4 k-tiles x 6 d-chunks.
  - Diagonal exclusion is folded into the matmul: the diag-block chain
    accumulates one extra rank-128 term (-BIG*I) @ I so exp underflows
    to 0 on the diagonal -- no post-exp masking pass.
  - E = exp(S^T * (1/t)) on ScalarE (PSUM -> SBUF bf16), 1/t from input.
  - out_psum[q, 0:769] = sum_k E^T[k,q] * [V | ones][k, :] on PE; col 768
    is the softmax denominator (ones-column trick).
  - out = out_psum[:, 0:768] * reciprocal(out_psum[:, 768]) -> HBM bf16.
"""

import os
import sys

if "/opt/trn_rl_repo" not in sys.path:
    sys.path.insert(0, "/opt/trn_rl_repo")

import numpy as np
import ml_dtypes

import concourse.bass as bass
import concourse.bacc as bacc
import concourse.tile as tile
from concourse import mybir
from concourse.bass_utils import run_bass_kernel_spmd

B, N, D = 64, 1024, 768
NCORES = 8
BPC = B // NCORES  # batches per core
P = 128
NT = N // P   # 8 n-tiles (also k-tiles / q-tiles)
DJ = D // P   # 6 d-chunks
F32 = mybir.dt.float32
BF16 = mybir.dt.bfloat16
NEG_BIG = -60000.0  # exp((s + NEG_BIG)/t) == 0 in fp32->bf16


def build_program(bpc: int = BPC) -> bacc.Bacc:
    nc = bacc.Bacc(
        "TRN2",
        target_bir_lowering=False,
        debug=False,
        num_devices=NCORES,
        num_swdge_queues=4,
    )
    q_h = nc.dram_tensor("q", [bpc, N, D], BF16, kind="ExternalInput").ap()
    k_h = nc.dram_tensor("k", [bpc, N, D], BF16, kind="ExternalInput").ap()
    v_h = nc.dram_tensor("v", [bpc, N, D], BF16, kind="ExternalInput").ap()
    t_h = nc.dram_tensor("t", [1], F32, kind="ExternalInput").ap()
    # eyes[:, 0:128] = I, eyes[:, 128:256] = NEG_BIG * I (single startup DMA)
    eyes_h = nc.dram_tensor("eyes", [P, 2 * P], BF16, kind="ExternalInput").ap()
    o_h = nc.dram_tensor("o", [bpc, N, D], BF16, kind="ExternalOutput").ap()

    with tile.TileContext(nc) as tc:
        with (
            tc.tile_pool(name="const", bufs=1) as const,
            tc.tile_pool(name="vpool", bufs=2) as vpool,
            tc.tile_pool(name="tpose", bufs=2) as tpose,
            tc.tile_pool(name="epool", bufs=2) as epool,
            tc.tile_pool(name="opool", bufs=3) as opool,
            tc.tile_pool(name="small", bufs=8) as small,
            tc.tile_pool(name="ps_s", bufs=4, space="PSUM") as ps_s,
            tc.tile_pool(name="ps_o", bufs=2, space="PSUM") as ps_o,
        ):
            const_tiles = {}

            def load_consts():
                # eyes gates the first diag matmul -- put it on the sync
                # queue ahead of the transposes so the queue's FIFO order
                # matches consumption order. t (gpsimd, tiny) only gates
                # the first activation, which has slack.
                eyes_sb = const.tile([P, 2 * P], BF16)
                nc.sync.dma_start(out=eyes_sb, in_=eyes_h)
                t_bc = const.tile([P, 1], F32)
                nc.gpsimd.dma_start(out=t_bc, in_=t_h.to_broadcast((P, 1)))
                inv_t = const.tile([P, 1], F32)
                nc.vector.reciprocal(inv_t, t_bc)
                const_tiles.update(
                    eye=eyes_sb[:, 0:P], neye=eyes_sb[:, P : 2 * P], inv_t=inv_t
                )

            def load_batch(b):
                """Issue batch b's input DMAs: Q,K xbar-transposed straight
                from DRAM into d-major [P, DJ, N] tiles (K on the sync
                HWDGE queue, Q on the scalar one, so they run concurrently),
                V copied natural. Batch 0 is split into half-row granules so
                the PE can start on the first (kt 0-3, q 0-511) chains
                early; its consts are slotted in between."""
                if b == 0:
                    load_consts()
                qT = tpose.tile([P, DJ, N], BF16, tag="qT")
                kT = tpose.tile([P, DJ, N], BF16, tag="kT")
                nsplit = 2 if b == 0 else 1
                h = N // nsplit
                for i in range(nsplit):
                    rows = slice(i * h, (i + 1) * h)
                    # Both transposes stay on the sync queue: two xbar
                    # transposes in flight on different HWDGE queues race
                    # (xbar mode switches interact with in-flight DMAs) and
                    # corrupt the transposed tiles nondeterministically.
                    nc.sync.dma_start(
                        out=kT[:, :, rows], in_=k_h[b, rows, :], transpose=True
                    )
                    nc.sync.dma_start(
                        out=qT[:, :, rows], in_=q_h[b, rows, :], transpose=True
                    )
                return qT, kT

            def load_v(b):
                # Batch 0's V is issued from the scalar engine mid-phase-1:
                # the preceding activations fence its issue behind the
                # critical transposes (the scheduler would otherwise hoist
                # it and chain the transposes after its completion), and a
                # copy on the scalar HWDGE runs concurrently with the sync
                # queue's transposes. Later V loads go via gpsimd SWDGE: a
                # steady-state V copy sharing the sync queue measured ~20%
                # slower on every engine.
                v_sb = vpool.tile([P, NT, D + 1], BF16, tag="vsb")
                eng = nc.scalar if b == 0 else nc.gpsimd
                eng.dma_start(
                    out=v_sb[:, :, 0:D],
                    in_=v_h[b].rearrange("(nt p) d -> p nt d", p=P),
                )
                nc.vector.memset(v_sb[:, :, D : D + 1], 1.0)
                return v_sb

            # 1-deep software pipeline: batch b+1's DMA chain is issued
            # before batch b's compute in program order, so the DMA engines
            # stay packed while the PE works on batch b. V loads are issued
            # mid-phase-1 (they are only needed by phase 2): issuing them
            # with the transposes makes the scheduler's serialized DMA
            # ordering delay the critical-path transposes behind them.
            pending = load_batch(0)
            v_pending = None
            for b in range(bpc):
                qT, kT = pending
                if b + 1 < bpc:
                    pending = load_batch(b + 1)

                # ---- S^T = K Q^T (k on partitions), diag fold, exp
                # Group order (kt 0-3 x half 0, kt 0-3 x half 1, kt 4-7 x
                # half 0, kt 4-7 x half 1) matches batch-0's half-row
                # transpose granules.
                ev = epool.tile([P, NT, N], BF16, tag="ev")
                for kh in range(2):
                    for half in range(2):
                        if kh == 0 and half == 1 and v_pending is None:
                            v_pending = load_v(b)  # b == 0 only
                        if kh == 1 and half == 0:
                            v_sb = v_pending
                        if kh == 1 and half == 1 and b + 1 < bpc:
                            v_pending = load_v(b + 1)
                        for kt in range(4 * kh, 4 * kh + 4):
                            sT = ps_s.tile([P, 512], F32, tag="sT")
                            is_diag = kt // 4 == half
                            for dj in range(DJ):
                                nc.tensor.matmul(
                                    sT,
                                    lhsT=kT[:, dj, kt * P : (kt + 1) * P],
                                    rhs=qT[:, dj, half * 512 : half * 512 + 512],
                                    start=(dj == 0),
                                    stop=(dj == DJ - 1 and not is_diag),
                                )
                            if is_diag:
                                # diag block: accumulate -BIG on the diagonal
                                c0 = (kt % 4) * P
                                nc.tensor.matmul(
                                    sT[:, c0 : c0 + P],
                                    lhsT=const_tiles["neye"],
                                    rhs=const_tiles["eye"],
                                    start=False,
                                    stop=True,
                                    skip_group_check=True,
                                )
                            nc.scalar.activation(
                                ev[:, kt, half * 512 : half * 512 + 512],
                                sT,
                                mybir.ActivationFunctionType.Exp,
                                scale=const_tiles["inv_t"],
                            )

                # ---- out = (E^T @ [V | 1]) then normalize by ones-column.
                # Two sequential accumulation chains per qt (cols 0:512 and
                # 512:769). Interleaving them per kt to reuse loaded weights
                # was measured SLOWER (+68us Tensor busy) -- alternating
                # PSUM banks per matmul breaks the PE's MM/LDW pipelining.
                for qt in range(NT):
                    o_ps = ps_o.tile([P, D + 1], F32, tag="o_ps")
                    for kt in range(NT):
                        nc.tensor.matmul(
                            o_ps[:, 0:512],
                            lhsT=ev[:, kt, qt * P : (qt + 1) * P],
                            rhs=v_sb[:, kt, 0:512],
                            start=(kt == 0),
                            stop=(kt == NT - 1),
                        )
                    for kt in range(NT):
                        nc.tensor.matmul(
                            o_ps[:, 512 : D + 1],
                            lhsT=ev[:, kt, qt * P : (qt + 1) * P],
                            rhs=v_sb[:, kt, 512 : D + 1],
                            start=(kt == 0),
                            stop=(kt == NT - 1),
                        )
                    rs = small.tile([P, 1], F32, tag="rs")
                    nc.vector.reciprocal(rs, o_ps[:, D : D + 1])
                    # 2-qt store granule in steady state (fine-grained
                    # stores lengthen the scheduler's serialized DMA chain
                    # and stall the next batch's transposes); per-qt on the
                    # last batch to shorten the drain tail.
                    gran = 1 if b == bpc - 1 else 2
                    if qt % gran == 0:
                        o_sb = opool.tile(
                            [P, gran, D], BF16, tag=f"o_sb{gran}"
                        )
                    nc.vector.tensor_scalar_mul(
                        o_sb[:, qt % gran, :], o_ps[:, 0:D], rs
                    )
                    if qt % gran == gran - 1:
                        q0 = (qt - gran + 1) * P
                        nc.gpsimd.dma_start(
                            out=o_h[b, q0 : (qt + 1) * P, :].rearrange(
                                "(j p) d -> p j d", p=P
                            ),
                            in_=o_sb,
                        )
    nc.finalize()
    return nc


_prog_cache: dict[int, bacc.Bacc] = {}


def _get_program(bpc: int) -> bacc.Bacc:
    if bpc not in _prog_cache:
        _prog_cache[bpc] = build_program(bpc)
    return _prog_cache[bpc]


def _run(Q, K, V, temperature, bpc: int = BPC, trace: bool = False):
    nc = _get_program(bpc)
    eyes = np.concatenate(
        [np.eye(P, dtype=np.float32), NEG_BIG * np.eye(P, dtype=np.float32)],
        axis=1,
    ).astype(ml_dtypes.bfloat16)
    t = np.asarray(temperature, dtype=np.float32).reshape(1)
    Qb = np.asarray(Q, dtype=np.float32).astype(ml_dtypes.bfloat16)
    Kb = np.asarray(K, dtype=np.float32).astype(ml_dtypes.bfloat16)
    Vb = np.asarray(V, dtype=np.float32).astype(ml_dtypes.bfloat16)
    in_maps = []
    for c in range(NCORES):
        sl = slice(c * bpc, (c + 1) * bpc)
        in_maps.append(
            {
                "q": np.ascontiguousarray(Qb[sl]),
                "k": np.ascontiguousarray(Kb[sl]),
                "v": np.ascontiguousarray(Vb[sl]),
                "t": t,
                "eyes": eyes,
            }
        )
    res = run_bass_kernel_spmd(
        nc, in_maps, core_ids=list(range(NCORES)), trace=trace
    )
    out = np.concatenate(
        [np.asarray(r["o"], dtype=np.float32) for r in res.results], axis=0
    )
    return out, res


def kernel(Q, K, V, temperature):
    # If BASS_TRACE leaked into the environment, the trace path would need
    # antenv.axon_hooks (absent in this image) and crash; force it off for
    # the plain grading path.
    if os.environ.get("BASS_TRACE"):
        try:
            import antenv.axon_hooks  # noqa: F401
        except ImportError:
            os.environ.pop("BASS_TRACE", None)
    out, _ = _run(Q, K, V, temperature)
    return out.astype(np.float32)


# revision 27
# speedup vs baseline: 1.0233x; 1.0233x over previous
"""Bass/Tile Trainium2 kernel for batched self-attention with diagonal
self-exclusion (LSA): out = softmax(mask_diag(Q K^T / t)) @ V.

Shapes: Q,K,V [64, 1024, 768] fp32, temperature [1] fp32.
Sharding: batch dim across 8 NeuronCores (8 batches/core, pure data parallel).

DMA-lean design (v2):
  - Host pre-casts Q,K,V to bf16 (the PE computes in bf16 anyway), halving
    HBM read traffic; output is stored bf16 and upcast on host.
  - Q,K are xbar DMA-transposed *directly from DRAM* into d-major SBUF
    tiles qT,kT [d, n] -- no staging tile, no SBUF->SBUF transpose pass.
  - S^T[k, q] = sum_d KT[d,k] * QT[d,q] on PE (bf16, fp32 PSUM accum),
    2 half-k groups x 2 q-halves x 4 k-tiles x 6 d-chunks.
  - Diagonal exclusion is folded into the matmul: the diag-block chain
    accumulates one extra rank-128 term (-BIG*I) @ I so exp underflows
    to 0 on the diagonal -- no post-exp masking pass.
  - E = exp(S^T * (1/t)) on ScalarE (PSUM -> SBUF bf16), 1/t from input.
  - out_psum[q, 0:769] = sum_k E^T[k,q] * [V | ones][k, :] on PE; col 768
    is the softmax denominator (ones-column trick).
  - out = out_psum[:, 0:768] * reciprocal(out_psum[:, 768]) -> HBM bf16.
"""

import os
import sys

if "/opt/trn_rl_repo" not in sys.path:
    sys.path.insert(0, "/opt/trn_rl_repo")

import numpy as np
import ml_dtypes

import concourse.bass as bass
import concourse.bacc as bacc
import concourse.tile as tile
from concourse import mybir
from concourse.bass_utils import run_bass_kernel_spmd

B, N, D = 64, 1024, 768
NCORES = 8
BPC = B // NCORES  # batches per core
P = 128
NT = N // P   # 8 n-tiles (also k-tiles / q-tiles)
DJ = D // P   # 6 d-chunks
F32 = mybir.dt.float32
BF16 = mybir.dt.bfloat16
NEG_BIG = -60000.0  # exp((s + NEG_BIG)/t) == 0 in fp32->bf16


def build_program(bpc: int = BPC) -> bacc.Bacc:
    nc = bacc.Bacc(
        "TRN2",
        target_bir_lowering=False,
        debug=False,
        num_devices=NCORES,
        num_swdge_queues=4,
    )
    q_h = nc.dram_tensor("q", [bpc, N, D], BF16, kind="ExternalInput").ap()
    k_h = nc.dram_tensor("k", [bpc, N, D], BF16, kind="ExternalInput").ap()
    v_h = nc.dram_tensor("v", [bpc, N, D], BF16, kind="ExternalInput").ap()
    t_h = nc.dram_tensor("t", [1], F32, kind="ExternalInput").ap()
    # eyes[:, 0:128] = I, eyes[:, 128:256] = NEG_BIG * I (single startup DMA)
    eyes_h = nc.dram_tensor("eyes", [P, 2 * P], BF16, kind="ExternalInput").ap()
    o_h = nc.dram_tensor("o", [bpc, N, D], BF16, kind="ExternalOutput").ap()

    with tile.TileContext(nc) as tc:
        with (
            tc.tile_pool(name="const", bufs=1) as const,
            tc.tile_pool(name="vpool", bufs=2) as vpool,
            tc.tile_pool(name="tpose", bufs=2) as tpose,
            tc.tile_pool(name="epool", bufs=2) as epool,
            tc.tile_pool(name="opool", bufs=3) as opool,
            tc.tile_pool(name="small", bufs=8) as small,
            tc.tile_pool(name="ps_s", bufs=4, space="PSUM") as ps_s,
            tc.tile_pool(name="ps_o", bufs=2, space="PSUM") as ps_o,
        ):
            const_tiles = {}

            def load_consts():
                # eyes gates the first diag matmul -- put it on the sync
                # queue ahead of the transposes so the queue's FIFO order
                # matches consumption order. t (gpsimd, tiny) only gates
                # the first activation, which has slack.
                eyes_sb = const.tile([P, 2 * P], BF16)
                nc.sync.dma_start(out=eyes_sb, in_=eyes_h)
                t_bc = const.tile([P, 1], F32)
                nc.gpsimd.dma_start(out=t_bc, in_=t_h.to_broadcast((P, 1)))
                inv_t = const.tile([P, 1], F32)
                nc.vector.reciprocal(inv_t, t_bc)
                const_tiles.update(
                    eye=eyes_sb[:, 0:P], neye=eyes_sb[:, P : 2 * P], inv_t=inv_t
                )

            def load_batch(b):
                """Issue batch b's input DMAs: Q,K xbar-transposed straight
                from DRAM into d-major [P, DJ, N] tiles (K on the sync
                HWDGE queue, Q on the scalar one, so they run concurrently),
                V copied natural. Batch 0 is split into half-row granules so
                the PE can start on the first (kt 0-3, q 0-511) chains
                early; its consts are slotted in between."""
                if b == 0:
                    load_consts()
                qT = tpose.tile([P, DJ, N], BF16, tag="qT")
                kT = tpose.tile([P, DJ, N], BF16, tag="kT")
                nsplit = 2 if b == 0 else 1
                h = N // nsplit
                for i in range(nsplit):
                    rows = slice(i * h, (i + 1) * h)
                    # Both transposes stay on the sync queue: two xbar
                    # transposes in flight on different HWDGE queues race
                    # (xbar mode switches interact with in-flight DMAs) and
                    # corrupt the transposed tiles nondeterministically.
                    nc.sync.dma_start(
                        out=kT[:, :, rows], in_=k_h[b, rows, :], transpose=True
                    )
                    nc.sync.dma_start(
                        out=qT[:, :, rows], in_=q_h[b, rows, :], transpose=True
                    )
                return qT, kT

            def load_v(b):
                # Batch 0's V rides the sync queue directly behind batch
                # 0's transposes (same-queue FIFO is the only ordering the
                # scheduler respects -- on any other queue it hoists the
                # load and chains the critical transposes after its
                # completion). Later V loads go via gpsimd SWDGE: a
                # steady-state V copy sharing the sync queue measured ~20%
                # slower on every engine.
                v_sb = vpool.tile([P, NT, D + 1], BF16, tag="vsb")
                eng = nc.sync if b == 0 else nc.gpsimd
                eng.dma_start(
                    out=v_sb[:, :, 0:D],
                    in_=v_h[b].rearrange("(nt p) d -> p nt d", p=P),
                )
                nc.vector.memset(v_sb[:, :, D : D + 1], 1.0)
                return v_sb

            # 1-deep software pipeline: batch b+1's DMA chain is issued
            # before batch b's compute in program order, so the DMA engines
            # stay packed while the PE works on batch b. V loads are issued
            # mid-phase-1 (they are only needed by phase 2): issuing them
            # with the transposes makes the scheduler's serialized DMA
            # ordering delay the critical-path transposes behind them.
            pending = load_batch(0)
            v_pending = load_v(0)
            for b in range(bpc):
                qT, kT = pending
                if b + 1 < bpc:
                    pending = load_batch(b + 1)

                # ---- S^T = K Q^T (k on partitions), diag fold, exp
                # Group order (kt 0-3 x half 0, kt 0-3 x half 1, kt 4-7 x
                # half 0, kt 4-7 x half 1) matches batch-0's half-row
                # transpose granules.
                ev = epool.tile([P, NT, N], BF16, tag="ev")
                for kh in range(2):
                    for half in range(2):
                        if kh == 1 and half == 0:
                            v_sb = v_pending
                        if kh == 1 and half == 1 and b + 1 < bpc:
                            v_pending = load_v(b + 1)
                        for kt in range(4 * kh, 4 * kh + 4):
                            sT = ps_s.tile([P, 512], F32, tag="sT")
                            is_diag = kt // 4 == half
                            for dj in range(DJ):
                                nc.tensor.matmul(
                                    sT,
                                    lhsT=kT[:, dj, kt * P : (kt + 1) * P],
                                    rhs=qT[:, dj, half * 512 : half * 512 + 512],
                                    start=(dj == 0),
                                    stop=(dj == DJ - 1 and not is_diag),
                                )
                            if is_diag:
                                # diag block: accumulate -BIG on the diagonal
                                c0 = (kt % 4) * P
                                nc.tensor.matmul(
                                    sT[:, c0 : c0 + P],
                                    lhsT=const_tiles["neye"],
                                    rhs=const_tiles["eye"],
                                    start=False,
                                    stop=True,
                                    skip_group_check=True,
                                )
                            nc.scalar.activation(
                                ev[:, kt, half * 512 : half * 512 + 512],
                                sT,
                                mybir.ActivationFunctionType.Exp,
                                scale=const_tiles["inv_t"],
                            )

                # ---- out = (E^T @ [V | 1]) then normalize by ones-column.
                # Two sequential accumulation chains per qt (cols 0:512 and
                # 512:769). Interleaving them per kt to reuse loaded weights
                # was measured SLOWER (+68us Tensor busy) -- alternating
                # PSUM banks per matmul breaks the PE's MM/LDW pipelining.
                for qt in range(NT):
                    o_ps = ps_o.tile([P, D + 1], F32, tag="o_ps")
                    for kt in range(NT):
                        nc.tensor.matmul(
                            o_ps[:, 0:512],
                            lhsT=ev[:, kt, qt * P : (qt + 1) * P],
                            rhs=v_sb[:, kt, 0:512],
                            start=(kt == 0),
                            stop=(kt == NT - 1),
                        )
                    for kt in range(NT):
                        nc.tensor.matmul(
                            o_ps[:, 512 : D + 1],
                            lhsT=ev[:, kt, qt * P : (qt + 1) * P],
                            rhs=v_sb[:, kt, 512 : D + 1],
                            start=(kt == 0),
                            stop=(kt == NT - 1),
                        )
                    rs = small.tile([P, 1], F32, tag="rs")
                    nc.vector.reciprocal(rs, o_ps[:, D : D + 1])
                    # 2-qt store granule in steady state (fine-grained
                    # stores lengthen the scheduler's serialized DMA chain
                    # and stall the next batch's transposes); per-qt on the
                    # last batch to shorten the drain tail.
                    gran = 1 if b == bpc - 1 else 2
                    if qt % gran == 0:
                        o_sb = opool.tile(
                            [P, gran, D], BF16, tag=f"o_sb{gran}"
                        )
                    nc.vector.tensor_scalar_mul(
                        o_sb[:, qt % gran, :], o_ps[:, 0:D], rs
                    )
                    if qt % gran == gran - 1:
                        q0 = (qt - gran + 1) * P
                        nc.gpsimd.dma_start(
                            out=o_h[b, q0 : (qt + 1) * P, :].rearrange(
                                "(j p) d -> p j d", p=P
                            ),
                            in_=o_sb,
                        )
    nc.finalize()
    return nc


_prog_cache: dict[int, bacc.Bacc] = {}


def _get_program(bpc: int) -> bacc.Bacc:
    if bpc not in _prog_cache:
        _prog_cache[bpc] = build_program(bpc)
    return _prog_cache[bpc]


def _run(Q, K, V, temperature, bpc: int = BPC, trace: bool = False):
    nc = _get_program(bpc)
    eyes = np.concatenate(
        [np.eye(P, dtype=np.float32), NEG_BIG * np.eye(P, dtype=np.float32)],
        axis=1,
    ).astype(ml_dtypes.bfloat16)
    t = np.asarray(temperature, dtype=np.float32).reshape(1)
    Qb = np.asarray(Q, dtype=np.float32).astype(ml_dtypes.bfloat16)
    Kb = np.asarray(K, dtype=np.float32).astype(ml_dtypes.bfloat16)
    Vb = np.asarray(V, dtype=np.float32).astype(ml_dtypes.bfloat16)
    in_maps = []
    for c in range(NCORES):
        sl = slice(c * bpc, (c + 1) * bpc)
        in_maps.append(
            {
                "q": np.ascontiguousarray(Qb[sl]),
                "k": np.ascontiguousarray(Kb[sl]),
                "v": np.ascontiguousarray(Vb[sl]),
                "t": t,
                "eyes": eyes,
            }
        )
    res = run_bass_kernel_spmd(
        nc, in_maps, core_ids=list(range(NCORES)), trace=trace
    )
    out = np.concatenate(
        [np.asarray(r["o"], dtype=np.float32) for r in res.results], axis=0
    )
    return out, res


def kernel(Q, K, V, temperature):
    # If BASS_TRACE leaked into the environment, the trace path would need
    # antenv.axon_hooks (absent in this image) and crash; force it off for
    # the plain grading path.
    if os.environ.get("BASS_TRACE"):
        try:
            import antenv.axon_hooks  # noqa: F401
        except ImportError:
            os.environ.pop("BASS_TRACE", None)
    out, _ = _run(Q, K, V, temperature)
    return out.astype(np.float32)


# revision 29
# speedup vs baseline: 1.0279x; 1.0045x over previous
"""Bass/Tile Trainium2 kernel for batched self-attention with diagonal
self-exclusion (LSA): out = softmax(mask_diag(Q K^T / t)) @ V.

Shapes: Q,K,V [64, 1024, 768] fp32, temperature [1] fp32.
Sharding: batch dim across 8 NeuronCores (8 batches/core, pure data parallel).

DMA-lean design (v2):
  - Host pre-casts Q,K,V to bf16 (the PE computes in bf16 anyway), halving
    HBM read traffic; output is stored bf16 and upcast on host.
  - Q,K are xbar DMA-transposed *directly from DRAM* into d-major SBUF
    tiles qT,kT [d, n] -- no staging tile, no SBUF->SBUF transpose pass.
  - S^T[k, q] = sum_d KT[d,k] * QT[d,q] on PE (bf16, fp32 PSUM accum),
    2 half-k groups x 2 q-halves x 4 k-tiles x 6 d-chunks.
  - Diagonal exclusion is folded into the matmul: the diag-block chain
    accumulates one extra rank-128 term (-BIG*I) @ I so exp underflows
    to 0 on the diagonal -- no post-exp masking pass.
  - E = exp(S^T * (1/t)) on ScalarE (PSUM -> SBUF bf16), 1/t from input.
  - out_psum[q, 0:769] = sum_k E^T[k,q] * [V | ones][k, :] on PE; col 768
    is the softmax denominator (ones-column trick).
  - out = out_psum[:, 0:768] * reciprocal(out_psum[:, 768]) -> HBM bf16.
"""

import os
import sys

if "/opt/trn_rl_repo" not in sys.path:
    sys.path.insert(0, "/opt/trn_rl_repo")

import numpy as np
import ml_dtypes

import concourse.bass as bass
import concourse.bacc as bacc
import concourse.tile as tile
from concourse import mybir
from concourse.bass_utils import run_bass_kernel_spmd

B, N, D = 64, 1024, 768
NCORES = 8
BPC = B // NCORES  # batches per core
P = 128
NT = N // P   # 8 n-tiles (also k-tiles / q-tiles)
DJ = D // P   # 6 d-chunks
F32 = mybir.dt.float32
BF16 = mybir.dt.bfloat16
NEG_BIG = -60000.0  # exp((s + NEG_BIG)/t) == 0 in fp32->bf16


def build_program(bpc: int = BPC) -> bacc.Bacc:
    nc = bacc.Bacc(
        "TRN2",
        target_bir_lowering=False,
        debug=False,
        num_devices=NCORES,
        num_swdge_queues=4,
    )
    q_h = nc.dram_tensor("q", [bpc, N, D], BF16, kind="ExternalInput").ap()
    k_h = nc.dram_tensor("k", [bpc, N, D], BF16, kind="ExternalInput").ap()
    v_h = nc.dram_tensor("v", [bpc, N, D], BF16, kind="ExternalInput").ap()
    t_h = nc.dram_tensor("t", [1], F32, kind="ExternalInput").ap()
    # eyes[:, 0:128] = I, eyes[:, 128:256] = NEG_BIG * I (single startup DMA)
    eyes_h = nc.dram_tensor("eyes", [P, 2 * P], BF16, kind="ExternalInput").ap()
    o_h = nc.dram_tensor("o", [bpc, N, D], BF16, kind="ExternalOutput").ap()

    with tile.TileContext(nc) as tc:
        with (
            tc.tile_pool(name="const", bufs=1) as const,
            tc.tile_pool(name="vpool", bufs=2) as vpool,
            tc.tile_pool(name="tpose", bufs=2) as tpose,
            tc.tile_pool(name="epool", bufs=2) as epool,
            tc.tile_pool(name="opool", bufs=3) as opool,
            tc.tile_pool(name="small", bufs=8) as small,
            tc.tile_pool(name="ps_s", bufs=4, space="PSUM") as ps_s,
            tc.tile_pool(name="ps_o", bufs=2, space="PSUM") as ps_o,
        ):
            const_tiles = {}

            def load_consts():
                # eyes gates the first diag matmul -- put it on the sync
                # queue ahead of the transposes so the queue's FIFO order
                # matches consumption order. t (gpsimd, tiny) only gates
                # the first activation, which has slack.
                eyes_sb = const.tile([P, 2 * P], BF16)
                nc.sync.dma_start(out=eyes_sb, in_=eyes_h)
                t_bc = const.tile([P, 1], F32)
                nc.gpsimd.dma_start(out=t_bc, in_=t_h.to_broadcast((P, 1)))
                inv_t = const.tile([P, 1], F32)
                nc.vector.reciprocal(inv_t, t_bc)
                const_tiles.update(
                    eye=eyes_sb[:, 0:P], neye=eyes_sb[:, P : 2 * P], inv_t=inv_t
                )

            def load_batch(b):
                """Issue batch b's input DMAs: Q,K xbar-transposed straight
                from DRAM into d-major [P, DJ, N] tiles (K on the sync
                HWDGE queue, Q on the scalar one, so they run concurrently),
                V copied natural. Batch 0 is split into half-row granules so
                the PE can start on the first (kt 0-3, q 0-511) chains
                early; its consts are slotted in between."""
                if b == 0:
                    load_consts()
                qT = tpose.tile([P, DJ, N], BF16, tag="qT")
                kT = tpose.tile([P, DJ, N], BF16, tag="kT")
                nsplit = 2 if b == 0 else 1
                h = N // nsplit
                for i in range(nsplit):
                    rows = slice(i * h, (i + 1) * h)
                    # Both transposes stay on the sync queue: two xbar
                    # transposes in flight on different HWDGE queues race
                    # (xbar mode switches interact with in-flight DMAs) and
                    # corrupt the transposed tiles nondeterministically.
                    nc.sync.dma_start(
                        out=kT[:, :, rows], in_=k_h[b, rows, :], transpose=True
                    )
                    nc.sync.dma_start(
                        out=qT[:, :, rows], in_=q_h[b, rows, :], transpose=True
                    )
                return qT, kT

            def load_v(b):
                # Batch 0's V rides the sync queue directly behind batch
                # 0's transposes (same-queue FIFO is the only ordering the
                # scheduler respects -- on any other queue it hoists the
                # load and chains the critical transposes after its
                # completion). Later V loads go via gpsimd SWDGE: a
                # steady-state V copy sharing the sync queue measured ~20%
                # slower on every engine.
                v_sb = vpool.tile([P, NT, D + 1], BF16, tag="vsb")
                eng = nc.sync if b == 0 else nc.gpsimd
                eng.dma_start(
                    out=v_sb[:, :, 0:D],
                    in_=v_h[b].rearrange("(nt p) d -> p nt d", p=P),
                )
                nc.vector.memset(v_sb[:, :, D : D + 1], 1.0)
                return v_sb

            # 1-deep software pipeline: batch b+1's DMA chain is issued
            # before batch b's compute in program order, so the DMA engines
            # stay packed while the PE works on batch b. V loads are issued
            # mid-phase-1 (they are only needed by phase 2): issuing them
            # with the transposes makes the scheduler's serialized DMA
            # ordering delay the critical-path transposes behind them.
            pending = load_batch(0)
            v_pending = load_v(0)
            fence_sb = const.tile([P, 2], BF16)
            for b in range(bpc):
                qT, kT = pending
                if b == 0:
                    # gpsimd fence: a 1-element copy that reads the last
                    # column Q0b writes. It precedes V1 in the gpsimd
                    # queue's FIFO, so V1's 1.5MB transfer cannot be
                    # hoisted into the startup window where it would steal
                    # DMA engines from the critical batch-0 transposes
                    # (measured: first transpose 7.3us instead of 3.4us).
                    nc.gpsimd.dma_start(
                        out=fence_sb, in_=qT[:, DJ - 1, N - 2 : N]
                    )
                if b + 1 < bpc:
                    pending = load_batch(b + 1)

                # ---- S^T = K Q^T (k on partitions), diag fold, exp
                # Group order (kt 0-3 x half 0, kt 0-3 x half 1, kt 4-7 x
                # half 0, kt 4-7 x half 1) matches batch-0's half-row
                # transpose granules.
                ev = epool.tile([P, NT, N], BF16, tag="ev")
                for kh in range(2):
                    for half in range(2):
                        if kh == 1 and half == 0:
                            v_sb = v_pending
                        if kh == 1 and half == 1 and b + 1 < bpc:
                            v_pending = load_v(b + 1)
                        for kt in range(4 * kh, 4 * kh + 4):
                            sT = ps_s.tile([P, 512], F32, tag="sT")
                            is_diag = kt // 4 == half
                            for dj in range(DJ):
                                nc.tensor.matmul(
                                    sT,
                                    lhsT=kT[:, dj, kt * P : (kt + 1) * P],
                                    rhs=qT[:, dj, half * 512 : half * 512 + 512],
                                    start=(dj == 0),
                                    stop=(dj == DJ - 1 and not is_diag),
                                )
                            if is_diag:
                                # diag block: accumulate -BIG on the diagonal
                                c0 = (kt % 4) * P
                                nc.tensor.matmul(
                                    sT[:, c0 : c0 + P],
                                    lhsT=const_tiles["neye"],
                                    rhs=const_tiles["eye"],
                                    start=False,
                                    stop=True,
                                    skip_group_check=True,
                                )
                            nc.scalar.activation(
                                ev[:, kt, half * 512 : half * 512 + 512],
                                sT,
                                mybir.ActivationFunctionType.Exp,
                                scale=const_tiles["inv_t"],
                            )

                # ---- out = (E^T @ [V | 1]) then normalize by ones-column.
                # Two sequential accumulation chains per qt (cols 0:512 and
                # 512:769). Interleaving them per kt to reuse loaded weights
                # was measured SLOWER (+68us Tensor busy) -- alternating
                # PSUM banks per matmul breaks the PE's MM/LDW pipelining.
                for qt in range(NT):
                    o_ps = ps_o.tile([P, D + 1], F32, tag="o_ps")
                    for kt in range(NT):
                        nc.tensor.matmul(
                            o_ps[:, 0:512],
                            lhsT=ev[:, kt, qt * P : (qt + 1) * P],
                            rhs=v_sb[:, kt, 0:512],
                            start=(kt == 0),
                            stop=(kt == NT - 1),
                        )
                    for kt in range(NT):
                        nc.tensor.matmul(
                            o_ps[:, 512 : D + 1],
                            lhsT=ev[:, kt, qt * P : (qt + 1) * P],
                            rhs=v_sb[:, kt, 512 : D + 1],
                            start=(kt == 0),
                            stop=(kt == NT - 1),
                        )
                    rs = small.tile([P, 1], F32, tag="rs")
                    nc.vector.reciprocal(rs, o_ps[:, D : D + 1])
                    # 2-qt store granule in steady state (fine-grained
                    # stores lengthen the scheduler's serialized DMA chain
                    # and stall the next batch's transposes); per-qt on the
                    # last batch to shorten the drain tail.
                    gran = 1 if b == bpc - 1 else 2
                    if qt % gran == 0:
                        o_sb = opool.tile(
                            [P, gran, D], BF16, tag=f"o_sb{gran}"
                        )
                    nc.vector.tensor_scalar_mul(
                        o_sb[:, qt % gran, :], o_ps[:, 0:D], rs
                    )
                    if qt % gran == gran - 1:
                        q0 = (qt - gran + 1) * P
                        # Last batch stores via the scalar HWDGE (idle by
                        # then) so the gpsimd queue drains early at
                        # teardown instead of costing ~4.6us.
                        seng = nc.scalar if b == bpc - 1 else nc.gpsimd
                        seng.dma_start(
                            out=o_h[b, q0 : (qt + 1) * P, :].rearrange(
                                "(j p) d -> p j d", p=P
                            ),
                            in_=o_sb,
                        )
    nc.finalize()
    return nc


_prog_cache: dict[int, bacc.Bacc] = {}


def _get_program(bpc: int) -> bacc.Bacc:
    if bpc not in _prog_cache:
        _prog_cache[bpc] = build_program(bpc)
    return _prog_cache[bpc]


def _run(Q, K, V, temperature, bpc: int = BPC, trace: bool = False):
    nc = _get_program(bpc)
    eyes = np.concatenate(
        [np.eye(P, dtype=np.float32), NEG_BIG * np.eye(P, dtype=np.float32)],
        axis=1,
    ).astype(ml_dtypes.bfloat16)
    t = np.asarray(temperature, dtype=np.float32).reshape(1)
    Qb = np.asarray(Q, dtype=np.float32).astype(ml_dtypes.bfloat16)
    Kb = np.asarray(K, dtype=np.float32).astype(ml_dtypes.bfloat16)
    Vb = np.asarray(V, dtype=np.float32).astype(ml_dtypes.bfloat16)
    in_maps = []
    for c in range(NCORES):
        sl = slice(c * bpc, (c + 1) * bpc)
        in_maps.append(
            {
                "q": np.ascontiguousarray(Qb[sl]),
                "k": np.ascontiguousarray(Kb[sl]),
                "v": np.ascontiguousarray(Vb[sl]),
                "t": t,
                "eyes": eyes,
            }
        )
    res = run_bass_kernel_spmd(
        nc, in_maps, core_ids=list(range(NCORES)), trace=trace
    )
    out = np.concatenate(
        [np.asarray(r["o"], dtype=np.float32) for r in res.results], axis=0
    )
    return out, res


def kernel(Q, K, V, temperature):
    # If BASS_TRACE leaked into the environment, the trace path would need
    # antenv.axon_hooks (absent in this image) and crash; force it off for
    # the plain grading path.
    if os.environ.get("BASS_TRACE"):
        try:
            import antenv.axon_hooks  # noqa: F401
        except ImportError:
            os.environ.pop("BASS_TRACE", None)
    out, _ = _run(Q, K, V, temperature)
    return out.astype(np.float32)
